# revision 7
# baseline (speedup 1.0000x reference)
"""LocalAttention3D Trainium2 kernel (v2.1 — Gram decomposition, PE tiling).

Problem: x [B=2, C=1, D=96, H=64, W=64], per-head scalar-affine q/k/v
projections (NH=4 heads), scores = einsum('bdjk,bdlm->bjklm', q, k)/sqrt(32),
softmax over the last W axis only (windows of 64), out = attn @ v, then sum
over heads.

Math: q.k decomposes over the scalar-affine projections:
  scores_h[jk,lm] = wq wk G[jk,lm] + wq bk s[jk] + bq wk s[lm] + D bq bk,
with G = X^T X (Gram) and s = X.sum(d).  The softmax over the window axis is
invariant to per-jk constants, so only logits = SCALE*(a_h G + c_h s_lm)
survive.  Both the per-head scale a_h and the per-partition bias c_h s_lm are
folded into an augmented 97-row contraction (XL = [a_h SCALE x; SCALE c_h s],
XR = [x; 1]), so the scalar engine runs a pure exp.  out_h = wv_h (P @ X^T)
+ 64 bv_h, with wv folded into the XT weights and the bias applied at evac.

Sharding: one (batch, head) pair per NeuronCore (2*4 = 8 cores), final head
sum on the host.

Per-core dataflow over 8 jk-strips of 512 (all lm-major, S^T layout);
concurrent PE tile pairs (col/row groups) double matmul throughput for the
MMZ / ZB / AV stages:
  phase A (lm-tile pairs (i, i+16)):
    MM-G: G_t[128, 512] = XL_tile^T @ XR_strip   (f32r, K=97 -> PSUM)
    ACT:  E_t = exp(G_t)                         (-> SBUF bf16)
    MMZ:  zf[0:32]/[32:64] += bones32_t^T @ E_t  (col-grp pair, accumulate)
  phase B: reciprocal -> zinvD [128, 512] bf16 (duplicated halves)
  phase C (per lm-tile t):
    MM-ZB: zb[0:64]   = bsel_t(top)^T @ zinvD[0:64]     (quadrant pair,
           zb[64:128] = bsel_t(bot)^T @ zinvD[64:128]    concurrent)
    DVE:   P_t = E_t * zb                        (bf16 * psum-f32)
    MM-AV: avA[96, 512] += XTW_t[0:64]^T @ P_t[0:64]    (split-K row pair,
           avB[96, 512] += XTW_t[64:128]^T @ P_t[64:128] concurrent)
  evac: out = avA + avB + 64*bv (ACT copy + DVE scalar_tensor_tensor).
"""

import math
import sys

sys.path.insert(0, "/opt/trn_rl_repo")

import numpy as np
import ml_dtypes

import bass_rust
import concourse.bass as bass
import concourse.tile as tile
from concourse import mybir
from concourse.bass_utils import run_bass_kernel_spmd

BF16 = ml_dtypes.bfloat16

B, D, HW = 2, 96, 64 * 64
NH = 4
NCORES = 8
NT = HW // 128        # 32 lm-tiles of 128 partitions (2 softmax windows each)
STRIP = 512           # jk columns per strip
NS = HW // STRIP      # 8 strips
SCALE = 1.0 / math.sqrt(32.0)


def _split_excess_waits(nc, max_waits=1):
    """This container's walrus rejects instructions with >1 semaphore wait
    ("Too many sync wait commands"). Move extra waits onto no-op carriers
    inserted just before the instruction on the same engine."""
    ctr = 0
    for f in nc.m.functions:
        for blk in f.blocks:
            insts = blk.instructions
            out = []
            changed = False
            for ins in insts:
                try:
                    si = ins.sync_info
                except Exception:
                    si = None
                if si is not None and len(si.on_wait) > max_waits:
                    waits = list(si.on_wait)
                    for w in waits[:-max_waits]:
                        ctr += 1
                        nop = mybir.InstNoOp(
                            name=f"wsplit-{ctr}-{ins.name}", ins=[], outs=[])
                        nop.engine = ins.engine
                        nop.sync_info = bass_rust.SyncInfo(
                            on_wait=[w], on_update=[])
                        nc.register_instruction(nop, overwrite=True)
                        out.append(nop)
                        changed = True
                    ins.sync_info = bass_rust.SyncInfo(
                        on_wait=waits[-max_waits:], on_update=list(si.on_update))
                out.append(ins)
            if changed:
                blk.instructions = out


def _build_program():
    f32 = mybir.dt.float32
    f32r = mybir.dt.float32r
    bf16 = mybir.dt.bfloat16
    Exp = mybir.ActivationFunctionType.Exp
    add = mybir.AluOpType.add

    nc = bass.Bass("TRN2", target_bir_lowering=False, debug=False,
                   num_devices=1)
    xl_d = nc.dram_tensor("xl", [D + 1, HW], f32r, kind="ExternalInput").ap()
    xr_d = nc.dram_tensor("xr", [D + 1, HW], f32r, kind="ExternalInput").ap()
    xt_d = nc.dram_tensor("xt", [128, NT * D], bf16, kind="ExternalInput").ap()
    bo_d = nc.dram_tensor("bo", [128, NT * 32], bf16,
                          kind="ExternalInput").ap()
    bs_d = nc.dram_tensor("bs", [128, NT * 64], bf16,
                          kind="ExternalInput").ap()
    sc_d = nc.dram_tensor("sc", [128, 8], f32, kind="ExternalInput").ap()
    out_d = nc.dram_tensor("out", [D, HW], f32, kind="ExternalOutput").ap()

    with tile.TileContext(nc) as tc:
        with (
            tc.tile_pool(name="cn", bufs=1) as cn,
            tc.tile_pool(name="ew", bufs=72) as ew,
            tc.tile_pool(name="zn", bufs=4) as zn,
            tc.tile_pool(name="pt", bufs=4) as ptp,
            tc.tile_pool(name="ob", bufs=6) as obp,
            tc.tile_pool(name="ps_g", bufs=2, space="PSUM") as ps_g,
            tc.tile_pool(name="ps_z", bufs=2, space="PSUM") as ps_z,
            tc.tile_pool(name="ps_zb", bufs=2, space="PSUM") as ps_zb,
            tc.tile_pool(name="ps_av", bufs=1, space="PSUM") as ps_av,
        ):
            XL = cn.tile([D + 1, HW], f32r, tag="XL")
            XR = cn.tile([D + 1, HW], f32r, tag="XR")
            XT = cn.tile([128, NT * D], bf16, tag="XT")
            BO = cn.tile([128, NT * 32], bf16, tag="BO")
            BS = cn.tile([128, NT * 64], bf16, tag="BS")
            SC = cn.tile([128, 8], f32, tag="SC")
            nc.sync.dma_start(XL[:], xl_d[:])
            nc.sync.dma_start(XR[:], xr_d[:])
            nc.sync.dma_start(XT[:], xt_d[:])
            nc.sync.dma_start(BO[:], bo_d[:])
            nc.sync.dma_start(BS[:], bs_d[:])
            nc.sync.dma_start(SC[:], sc_d[:])

            # Software-pipelined: phase A of strip s+1 interleaves with
            # phase C of strip s at tile granularity; consumers lag their
            # producers by >=1 slot so the PE stream never stalls (keeps
            # HAM warm at 2.4 GHz).
            e_all = [[None] * NT for _ in range(NS)]
            zD_all = [None] * NS
            zf_all = [None] * NS

            def a_slots(s):
                """Phase A of strip s as 16+2 emission thunks."""
                j0 = s * STRIP
                zf = ps_z.tile([64, STRIP], f32, tag="zf")
                zf_all[s] = zf

                def mmz(i):
                    nc.tensor.matmul(
                        zf[0:32, :], BO[:, i * 32:(i + 1) * 32],
                        e_all[s][i][:], start=(i == 0), stop=(i == 15))
                    nc.tensor.matmul(
                        zf[32:64, :], BO[:, (i + 16) * 32:(i + 17) * 32],
                        e_all[s][i + 16][:], start=(i == 0), stop=(i == 15))

                def slot(i):
                    if i < 16:
                        for t in (i, i + 16):
                            g = ps_g.tile([128, STRIP], f32, tag="g")
                            nc.tensor.matmul(
                                g[:], XL[:, t * 128:(t + 1) * 128],
                                XR[:, j0:j0 + STRIP], start=True, stop=True)
                            et = ew.tile([128, STRIP], bf16, tag="et")
                            nc.scalar.activation(et[:], g[:], Exp)
                            e_all[s][t] = et
                    if i >= 2:
                        mmz(i - 2)
                return [lambda i=i: slot(i) for i in range(18)]

            def b_emit(s):
                """Phase B of strip s (DVE): reciprocal + bf16 dup halves."""
                zi = zn.tile([64, STRIP], f32, tag="zi")
                nc.vector.reciprocal(zi[:], zf_all[s][:])
                zD = zn.tile([128, STRIP], bf16, tag="zD")
                nc.vector.tensor_copy(zD[0:64, :], zi[:])
                nc.vector.tensor_copy(zD[64:128, :], zi[:])
                zD_all[s] = zD

            def c_tiles(s):
                """Phase C of strip s as 32+2 emission thunks."""
                j0 = s * STRIP
                avA = ps_av.tile([D, STRIP], f32, tag="avA")
                avB = ps_av.tile([D, STRIP], f32, tag="avB")
                zbs = [None] * NT
                pts = [None] * NT

                def tilefn(k):
                    zD = zD_all[s]
                    if k < NT:
                        zb = ps_zb.tile([128, STRIP], f32, tag="zb")
                        nc.tensor.matmul(
                            zb[0:64, :], BS[0:64, k * 64:(k + 1) * 64],
                            zD[0:64, :], start=True, stop=True)
                        nc.tensor.matmul(
                            zb[64:128, :], BS[64:128, k * 64:(k + 1) * 64],
                            zD[64:128, :], start=True, stop=True)
                        zbs[k] = zb
                    if 1 <= k <= NT:
                        pt = ptp.tile([128, STRIP], bf16, tag="pt")
                        nc.vector.tensor_mul(
                            pt[:], e_all[s][k - 1][:], zbs[k - 1][:])
                        pts[k - 1] = pt
                    if k >= 2:
                        t = k - 2
                        nc.tensor.matmul(
                            avA[:], XT[0:64, t * D:(t + 1) * D],
                            pts[t][0:64, :],
                            start=(t == 0), stop=(t == NT - 1))
                        nc.tensor.matmul(
                            avB[:], XT[64:128, t * D:(t + 1) * D],
                            pts[t][64:128, :],
                            start=(t == 0), stop=(t == NT - 1))
                    if k == NT + 1:
                        o2 = obp.tile([D, STRIP], f32, tag="o2")
                        nc.scalar.copy(o2[:], avB[:])
                        ob = obp.tile([D, STRIP], f32, tag="ob")
                        nc.vector.scalar_tensor_tensor(
                            ob[:], avA[:], SC[:D, 2:3], o2[:], add, add)
                        nc.sync.dma_start(out_d[:, j0:j0 + STRIP], ob[:])
                return [lambda k=k: tilefn(k) for k in range(NT + 2)]

            for s in range(NS + 1):
                a = a_slots(s) if s < NS else []
                c = c_tiles(s - 1) if s >= 1 else []
                # front-load a few A slots to cover strip s-1's reciprocal
                nfront = min(5, len(a)) if c else len(a)
                for th in a[:nfront]:
                    th()
                rest_a = a[nfront:]
                # interleave: 2 C tiles per remaining A slot
                ia = 0
                for k, th in enumerate(c):
                    th()
                    if k % 2 == 1 and ia < len(rest_a):
                        rest_a[ia]()
                        ia += 1
                for th in rest_a[ia:]:
                    th()
                if s < NS:
                    b_emit(s)

    _split_excess_waits(nc)
    return nc


_NC = None


def _get_program():
    global _NC
    if _NC is None:
        _NC = _build_program()
    return _NC


def _make_in_maps(x, wq, bq, wk, bk, wv, bv):
    x = np.asarray(x, dtype=np.float32)
    x2 = x.reshape(B, D, HW)
    wq, bq, wk, bk, wv, bv = [
        np.asarray(a, dtype=np.float32) for a in (wq, bq, wk, bk, wv, bv)]
    ssum = x2.sum(axis=1)  # [B, HW] — s_lm = sum_d x[d, lm]

    # MMZ weights (paired col groups): block t [128, 32]:
    #   p<64 -> col 2t mod 32, p>=64 -> col (2t+1) mod 32
    bones = np.zeros((128, NT * 32), dtype=BF16)
    for t in range(NT):
        bones[0:64, t * 32 + (2 * t) % 32] = BF16(1.0)
        bones[64:128, t * 32 + (2 * t + 1) % 32] = BF16(1.0)

    # ZB selector (quadrant pairs): block t [128, 64]:
    #   top: row 2t ones (out parts 0-63), bottom: row 64+2t+1 (out 64-127)
    bsel = np.zeros((128, NT * 64), dtype=BF16)
    for t in range(NT):
        bsel[2 * t, t * 64:(t + 1) * 64] = BF16(1.0)
        bsel[64 + 2 * t + 1, t * 64:(t + 1) * 64] = BF16(1.0)

    in_maps = []
    for c in range(NCORES):
        b, h = divmod(c, NH)
        xb = x2[b]
        a_h = wq[h] * wk[h]
        c_h = bq[h] * wk[h]
        xl = np.empty((D + 1, HW), dtype=np.float32)
        xl[0:D] = SCALE * a_h * xb
        xl[D] = SCALE * c_h * ssum[b]
        xr = np.empty((D + 1, HW), dtype=np.float32)
        xr[0:D] = xb
        xr[D] = 1.0
        xt = np.ascontiguousarray(
            (wv[h] * xb).reshape(D, NT, 128).transpose(2, 1, 0)
            .reshape(128, NT * D)).astype(BF16)
        sc = np.zeros((128, 8), dtype=np.float32)
        sc[:, 2] = 64.0 * bv[h]            # evac bias
        in_maps.append({
            "xl": xl,
            "xr": xr,
            "xt": xt,
            "bo": bones,
            "bs": bsel,
            "sc": sc,
        })
    return in_maps


def kernel(x, wq, bq, wk, bk, wv, bv):
    nc = _get_program()
    in_maps = _make_in_maps(x, wq, bq, wk, bk, wv, bv)
    res = run_bass_kernel_spmd(nc, in_maps, core_ids=list(range(NCORES)))
    out = np.zeros((B, 1, D, 64, 64), dtype=np.float32)
    for c in range(NCORES):
        b = c // NH
        out[b, 0] += res.results[c]["out"].reshape(D, 64, 64)
    return out
